# revision 2
# baseline (speedup 1.0000x reference)
"""Trainium2 Bass kernel for nn_AttentionBlock (B=8, C=256, H=W=64).

Data-parallel over batch: each of the 8 NeuronCores processes one [C, N]
slab (N = H*W = 4096).

Per-core math (all layouts partition-major):
  x      [C, N]   (c on partitions, 2 chunks of 128)
  k      = Wk @ x + bk          [C, N]  (o on partitions)   f32r
  vT     = x^T @ Wv^T + bv      [N, C]  (m on partitions)   f32r
  per n-panel (512 cols):
    q    = Wq @ x[:, panel] + bq        [C, 512]            f32r
    proj = Wp @ x[:, panel] + bp        [C, 512]            f32
    ST[m, n] = sum_c k[c, m] q[c, n]    (attn^T, m on partitions)
    E = exp(ST / 16)                    [4096, 512]         f32r
    colsum[n] = sum_m E[m, n]           (ones-matmul over partitions)
    Hu[c, n] = sum_m vT[m, c] E[m, n]   (PE, accum 32 m-tiles)
    out = Hu / colsum + proj
Softmax skips max-subtraction: logits ~N(0,1) after the 1/16 scale, so
exp stays in a safe fp32 range.
"""
import sys

if "/opt/trn_rl_repo" not in sys.path:
    sys.path.insert(0, "/opt/trn_rl_repo")

import numpy as np

import concourse.bacc as bacc
import concourse.mybir as mybir
import concourse.tile as tile
from concourse.bass_utils import run_bass_kernel_spmd

F32 = mybir.dt.float32
F32R = mybir.dt.float32r
EXP = mybir.ActivationFunctionType.Exp
ADD = mybir.AluOpType.add
MULT = mybir.AluOpType.mult

B, C, H, W = 8, 256, 64, 64
N = H * W            # 4096
PN = 512             # panel width (n columns per panel)
NPANELS = N // PN    # 8
NMT = N // 128       # 32 m-tiles
SCALE = float(C) ** -0.5


def build_nc():
    nc = bacc.Bacc()
    x_d = nc.dram_tensor("x", [C, N], F32, kind="ExternalInput")
    w_d = {}
    b_d = {}
    for nm in ("wq", "wk", "wv", "wp"):
        w_d[nm] = nc.dram_tensor(nm + "t", [C, C], F32, kind="ExternalInput")
    for nm in ("bq", "bk", "bv", "bp"):
        b_d[nm] = nc.dram_tensor(nm, [C], F32, kind="ExternalInput")
    out_d = nc.dram_tensor("out", [C, N], F32, kind="ExternalOutput")

    with tile.TileContext(nc) as tc:
        with (
            tc.tile_pool(name="consts", bufs=1) as consts,
            tc.tile_pool(name="wstage", bufs=2) as wstage,
            tc.tile_pool(name="big", bufs=1) as big,
            tc.tile_pool(name="xchunk", bufs=2) as xchunk,
            tc.tile_pool(name="panel", bufs=2) as panel,
            tc.tile_pool(name="outs", bufs=4) as outs,
            tc.tile_pool(name="ps_qk", bufs=2, space="PSUM") as ps_qk,
            tc.tile_pool(name="ps_h", bufs=2, space="PSUM") as ps_h,
            tc.tile_pool(name="ps_sum", bufs=1, space="PSUM") as ps_sum,
            tc.tile_pool(name="ps_rb", bufs=1, space="PSUM") as ps_rb,
        ):
            # ---- constants: weights (transposed on host), biases, ones ----
            w_r = {}
            for nm in ("wq", "wk", "wv", "wp"):
                w_in = wstage.tile([128, 2, C], F32, tag="w_in", name=f"{nm}_in")
                nc.sync.dma_start(
                    out=w_in, in_=w_d[nm].rearrange("(ct p) o -> p ct o", p=128))
                w_rt = consts.tile([128, 2, C], F32R, tag=f"{nm}_r", name=f"{nm}_r")
                nc.vector.tensor_copy(w_rt[:], w_in[:])
                w_r[nm] = w_rt

            bias_t = {}
            for nm in ("bq", "bk", "bp"):
                bt = consts.tile([128, 2], F32, tag=f"{nm}_t", name=f"{nm}_t")
                nc.sync.dma_start(out=bt, in_=b_d[nm].rearrange("(oc p) -> p oc", p=128))
                bias_t[nm] = bt
            bv_row_f = consts.tile([1, C], F32)
            nc.sync.dma_start(out=bv_row_f, in_=b_d["bv"][None, :])
            bv_row = consts.tile([1, C], F32R)
            nc.vector.tensor_copy(bv_row[:], bv_row_f[:])

            ones1_f = consts.tile([1, 128], F32)
            nc.vector.memset(ones1_f[:], 1.0)
            ones1 = consts.tile([1, 128], F32R)
            nc.vector.tensor_copy(ones1[:], ones1_f[:])
            ones128_f = consts.tile([128, 1], F32)
            nc.vector.memset(ones128_f[:], 1.0)
            ones128 = consts.tile([128, 1], F32R)
            nc.vector.tensor_copy(ones128[:], ones128_f[:])

            # bvb[p, c] = bv[c] broadcast over partitions (K=1 ones matmul)
            ps_bvb = ps_rb.tile([128, C], F32, tag="rb")
            nc.tensor.matmul(ps_bvb[:], ones1[:], bv_row[:], start=True, stop=True)
            bvb = consts.tile([128, C], F32)
            nc.vector.tensor_copy(bvb[:], ps_bvb[:])

            # ---- persistent per-batch tensors ----
            k_r = big.tile([128, 2, N], F32R, tag="k_r")       # [oc, n]
            vt_r = big.tile([128, NMT, C], F32R, tag="vt_r")   # [mt, c]
            e_r = big.tile([128, NMT, PN], F32R, tag="e_r")    # [mt, n]

            def load_x_chunk(j, tag_suffix=""):
                x_in = xchunk.tile([128, 2, PN], F32, tag="x_in")
                nc.sync.dma_start(
                    out=x_in,
                    in_=x_d.rearrange("(ct p) n -> p ct n", p=128)[
                        :, :, j * PN:(j + 1) * PN])
                x_r = xchunk.tile([128, 2, PN], F32R, tag="x_r")
                nc.vector.tensor_copy(x_r[:], x_in[:])
                return x_r

            # ---- pass 0: k and vT over 8 x-chunks ----
            for j in range(NPANELS):
                x_r = load_x_chunk(j)
                ps_k = ps_qk.tile([128, 2, PN], F32, tag="qk", name=f"ps_k{j}")
                for oc in range(2):
                    for ct in range(2):
                        nc.tensor.matmul(
                            ps_k[:, oc, :],
                            w_r["wk"][:, ct, oc * 128:(oc + 1) * 128],
                            x_r[:, ct, :],
                            start=(ct == 0), stop=(ct == 1))
                for oc in range(2):
                    nc.vector.tensor_scalar(
                        out=k_r[:, oc, j * PN:(j + 1) * PN],
                        in0=ps_k[:, oc, :],
                        scalar1=bias_t["bk"][:, oc:oc + 1],
                        scalar2=None, op0=ADD)
                ps_v = ps_qk.tile([128, 4, C], F32, tag="qk", name=f"ps_v{j}")
                for ml in range(4):
                    for ct in range(2):
                        nc.tensor.matmul(
                            ps_v[:, ml, :],
                            x_r[:, ct, ml * 128:(ml + 1) * 128],
                            w_r["wv"][:, ct, :],
                            start=(ct == 0), stop=(ct == 1))
                for ml in range(4):
                    nc.vector.tensor_tensor(
                        out=vt_r[:, 4 * j + ml, :],
                        in0=ps_v[:, ml, :], in1=bvb[:], op=ADD)

            # ---- panels ----
            for p in range(NPANELS):
                n0 = p * PN
                x_r = load_x_chunk(p)
                ps_q = ps_qk.tile([128, 2, PN], F32, tag="qk", name=f"ps_q{p}")
                for oc in range(2):
                    for ct in range(2):
                        nc.tensor.matmul(
                            ps_q[:, oc, :],
                            w_r["wq"][:, ct, oc * 128:(oc + 1) * 128],
                            x_r[:, ct, :],
                            start=(ct == 0), stop=(ct == 1))
                q_r = panel.tile([128, 2, PN], F32R, tag="q_r")
                for oc in range(2):
                    nc.vector.tensor_scalar(
                        out=q_r[:, oc, :], in0=ps_q[:, oc, :],
                        scalar1=bias_t["bq"][:, oc:oc + 1],
                        scalar2=None, op0=ADD)
                ps_p = ps_qk.tile([128, 2, PN], F32, tag="qk", name=f"ps_p{p}")
                for oc in range(2):
                    for ct in range(2):
                        nc.tensor.matmul(
                            ps_p[:, oc, :],
                            w_r["wp"][:, ct, oc * 128:(oc + 1) * 128],
                            x_r[:, ct, :],
                            start=(ct == 0), stop=(ct == 1))
                proj_t = panel.tile([128, 2, PN], F32, tag="proj_t")
                for oc in range(2):
                    nc.vector.tensor_scalar(
                        out=proj_t[:, oc, :], in0=ps_p[:, oc, :],
                        scalar1=bias_t["bp"][:, oc:oc + 1],
                        scalar2=None, op0=ADD)

                # QK^T (transposed layout) + exp, two m-tiles per psum
                for mth in range(NMT // 2):
                    ps_s = ps_qk.tile([128, 2, PN], F32, tag="qk",
                                      name=f"ps_s{p}_{mth}")
                    for sub in range(2):
                        mt = 2 * mth + sub
                        for ct in range(2):
                            nc.tensor.matmul(
                                ps_s[:, sub, :],
                                k_r[:, ct, mt * 128:(mt + 1) * 128],
                                q_r[:, ct, :],
                                start=(ct == 0), stop=(ct == 1))
                    nc.scalar.activation(
                        out=e_r[:, 2 * mth:2 * mth + 2, :], in_=ps_s[:],
                        func=EXP, scale=SCALE)

                # A @ V and column sums, accumulating over 32 m-tiles
                ps_h0 = ps_h.tile([128, PN], F32, tag="h", name=f"ps_h0_{p}")
                ps_h1 = ps_h.tile([128, PN], F32, tag="h", name=f"ps_h1_{p}")
                ps_cs = ps_sum.tile([1, PN], F32, tag="cs", name=f"ps_cs{p}")
                for mt in range(NMT):
                    st, sp = (mt == 0), (mt == NMT - 1)
                    nc.tensor.matmul(ps_h0[:], vt_r[:, mt, 0:128],
                                     e_r[:, mt, :], start=st, stop=sp)
                    nc.tensor.matmul(ps_h1[:], vt_r[:, mt, 128:256],
                                     e_r[:, mt, :], start=st, stop=sp)
                    nc.tensor.matmul(ps_cs[0:1, :], ones128[:, 0:1],
                                     e_r[:, mt, :], start=st, stop=sp)

                # normalize + residual
                r_row = panel.tile([1, PN], F32R, tag="r_row")
                with nc.allow_low_precision(reason="f32r rounding of softmax recip"):
                    nc.vector.reciprocal(out=r_row[:], in_=ps_cs[0:1, :])
                ps_r = ps_rb.tile([128, PN], F32, tag="rb", name=f"ps_r{p}")
                nc.tensor.matmul(ps_r[:], ones1[:], r_row[:], start=True, stop=True)
                rb = panel.tile([128, PN], F32, tag="rb_t")
                nc.vector.tensor_copy(rb[:], ps_r[:])
                for oc, ph in ((0, ps_h0), (1, ps_h1)):
                    out_t = outs.tile([128, PN], F32, tag="out_t")
                    nc.vector.tensor_tensor(out=out_t[:], in0=ph[:], in1=rb[:],
                                            op=MULT)
                    nc.vector.tensor_tensor(out=out_t[:], in0=out_t[:],
                                            in1=proj_t[:, oc, :], op=ADD)
                    nc.sync.dma_start(
                        out=out_d[oc * 128:(oc + 1) * 128, n0:n0 + PN],
                        in_=out_t[:])

    nc.compile()
    return nc


_NC = None


def kernel(**inputs) -> np.ndarray:
    global _NC
    x = np.asarray(inputs["x"], dtype=np.float32)
    ws = {nm: np.ascontiguousarray(np.asarray(inputs[nm], np.float32).T)
          for nm in ("Wq", "Wk", "Wv", "Wp")}
    bs = {nm: np.asarray(inputs[nm], np.float32)
          for nm in ("bq", "bk", "bv", "bp")}
    if _NC is None:
        _NC = build_nc()
    base = {
        "wqt": ws["Wq"], "wkt": ws["Wk"], "wvt": ws["Wv"], "wpt": ws["Wp"],
        "bq": bs["bq"], "bk": bs["bk"], "bv": bs["bv"], "bp": bs["bp"],
    }
    in_maps = [dict(base, x=np.ascontiguousarray(x[b].reshape(C, N)))
               for b in range(B)]
    res = run_bass_kernel_spmd(_NC, in_maps, core_ids=list(range(B)))
    out = np.stack([res.results[b]["out"] for b in range(B)], axis=0)
    return out.reshape(B, C, H, W)
